# revision 24
# baseline (speedup 1.0000x reference)
"""Trainium2 Bass kernel for nn_ContractExpand (segment_reduce, 5 scales).

out[n, b, l, e] = relu(segsum_r(x)[b, g(l), :] @ (W[n]/r).T + b[n]/r)  broadcast over groups

Strategy (data-parallel over B across 8 cores, 8 batches each):
 - host: augment x with a ones-column plus zero pad, packed p-major as
   [B_LOC,100,8,304] so every load descriptor is 4864B contiguous; fold
   bias + the 1/r scale into WT_aug[n] = [W[n].T/r ; b[n]/r^2 ; 0 ; 0],
   shipped pre-transposed as [3,101,5,300] (3000B descriptors).
 - the PE stalls ~300ns whenever consecutive matmuls change the stationary's
   32-aligned row/col-group footprint (measured), so every matmul keeps
   K in {100,101} and P=100/101 (4x4 groups).  Strided stationary reads are
   free (measured), which the r=1 path uses to pair-interleave output rows.
 - device, per batch:
     1. segsum-matmul: PE computes seg_augT[d, g] for ALL 5 scales at once
        into tile-major PSUM, two ping-pong halves (4 l-tiles each).
     2. de-scramble PSUM -> SBUF bf16 per (half, scale) on DVE/ACT/GpSimd.
     3. main matmul in P=100 g-tile windows (r=10/25 use overlapping
        windows so P stays 100); ReLU evacuates to SBUF bf16.
     4. stores: every descriptor >= 1200B of contiguous DRAM (sub-KB
        descriptors collapse SDMA throughput, measured):
          r=1  -> rows pair-interleaved into partitions via stride-2
                  stationary cols (2 consecutive rows per partition)
          r>=2 -> rho in-SBUF row replicas (rho=2, r=25: 5) written by
                  cheap bf16 copies; DMA broadcast covers only r/rho.
   Host upcasts bf16 -> f32.
"""

import numpy as np
import ml_dtypes

import concourse.bass as bass
import concourse.tile as tile
from concourse import bacc, mybir
from concourse.bass_utils import run_bass_kernel_spmd

F32 = mybir.dt.float32
BF16 = mybir.dt.bfloat16

R_SCALES = (1, 2, 4, 10, 25)
B, L, D = 64, 800, 300
NCORES = 8
B_LOC = B // NCORES          # 8 batches per core
LT = 100                     # l-tile rows; all scale group sizes align
NT = L // LT                 # 8 l-tiles
NH = 2                       # segsum psum halves (ping-pong)
TPH = NT // NH               # l-tiles per half
SCOLS = [LT // r for r in R_SCALES]                 # 100 50 25 10 4
SCOFF = np.cumsum([0] + SCOLS).tolist()             # s_pack col offsets
SC = SCOFF[-1]                                      # 189
SCP = 190                                           # padded (even) s_pack cols
G = [L // r for r in R_SCALES]                      # 800 400 200 80 32
POFF = np.cumsum([0] + G).tolist()                  # packed seg col offsets
GTOT = POFF[-1]                                     # 1512
GPAD = 4                     # seg pad cols so the r25 window stays in-tile
DSLICES = [(0, 101), (101, 101), (202, 101)]        # equal K-tiles of x_aug
PBLK = 256                                          # psum cols per l-tile block
FLUSH = [3, 3, 3, 3, 2, 2]   # units flushed after each of 6 segsum chunks


def build_s_pack():
    s = np.zeros((LT, NT, SCP), np.float32)
    for t in range(NT):
        for si, r in enumerate(R_SCALES):
            for p in range(LT):
                s[p, t, SCOFF[si] + p // r] = 1.0
    return s.astype(ml_dtypes.bfloat16)


def build_wt_aug(W, b):
    out = np.zeros((5, 303, D), np.float64)
    for n, r in enumerate(R_SCALES):
        out[n, :D, :] = np.asarray(W[n], np.float64).T / r
        out[n, D, :] = np.asarray(b[n], np.float64) / (r * r)
    # host layout matches the SBUF tile ([101, 5, 3, 300], scale-major) so
    # each per-scale load is a flat copy with 1800B-contiguous descriptors
    w3 = out.astype(ml_dtypes.bfloat16).transpose(1, 0, 2).reshape(3, 101, 5, D)
    return np.ascontiguousarray(w3.transpose(1, 2, 0, 3))


def _body(tc, out_ap, x_ap, wt_ap, spk_ap):
    nc = tc.nc
    with (
        tc.tile_pool(name="consts", bufs=1) as consts,
        tc.tile_pool(name="xp", bufs=3) as xp,
        tc.tile_pool(name="segp", bufs=2) as segp,
        tc.tile_pool(name="yp", bufs=3) as yp,
        tc.tile_pool(name="psp", bufs=2, space="PSUM") as psp,
        tc.tile_pool(name="mpsp", bufs=4, space="PSUM") as mpsp,
    ):
        spk_sb = consts.tile([LT, NT, SCP], BF16, name="spk_sb")
        nc.gpsimd.dma_start(out=spk_sb[:, :, :], in_=spk_ap[:, :, :])
        wall = consts.tile([101, 5, 3, D], BF16, name="wall")
        wtiles = [[wall[:, n, k, :] for k in range(3)] for n in range(5)]

        def load_x(b, split=False):
            # all loads ride the gpsimd (SWDGE) ring in need-order; the first
            # load is split so the h=0 segsum chunks start after half the data
            x_sb = xp.tile([LT, NT, 304], BF16, name="x_sb", tag="x")
            if split:
                nc.gpsimd.dma_start(out=x_sb[:, 0:TPH, :], in_=x_ap[b][:, 0:TPH, :])
                nc.scalar.dma_start(out=x_sb[:, TPH:NT, :], in_=x_ap[b][:, TPH:NT, :])
            else:
                nc.gpsimd.dma_start(out=x_sb[:, :, :], in_=x_ap[b][:, :, :])
            return x_sb

        def segsum_chunk(x_sb, segs, k, h):
            """Segsum matmuls for l-tiles [4h, 4h+4) of K-tile k, then the
            de-scramble copies into segs[k]."""
            d0, dw = DSLICES[k]
            ps = psp.tile([101, TPH * PBLK], F32, name="segps", tag="segps")
            for i in range(TPH):
                t = TPH * h + i
                dst = PBLK * i
                nc.tensor.matmul(
                    ps[0:dw, dst : dst + SC],
                    x_sb[:, t, d0 : d0 + dw],
                    spk_sb[:, t, 0:SC],
                    start=(i % 2 == 0),
                    stop=(i % 2 == 1),
                )
            pst = ps[0:dw, :].rearrange("p (t c) -> p t c", c=PBLK)
            for si in range(5):
                w_ = SCOLS[si]
                src = pst[:, :, SCOFF[si] : SCOFF[si] + w_]
                c0 = POFF[si] + h * TPH * w_
                dst_ = segs[k][0:dw, c0 : c0 + TPH * w_].rearrange(
                    "p (t c) -> p t c", t=TPH
                )
                # balance: big copy on DVE, the rest on ACT
                if si in (0, 4):
                    nc.vector.tensor_copy(dst_, src)
                else:
                    nc.scalar.copy(dst_, src)

        ecnt = [0]

        def eng_op(fn_dve, fn_act):
            if ecnt[0] % 2 == 0:
                fn_dve()
            else:
                fn_act()
            ecnt[0] += 1

        def relu_to(y_slice, mp):
            eng_op(
                lambda: nc.vector.tensor_scalar_max(y_slice, mp, 0.0),
                lambda: nc.scalar.activation(
                    y_slice, mp, mybir.ActivationFunctionType.Relu
                ),
            )

        def copy_to(dst, src):
            # replica copies are small SBUF->SBUF bf16 moves: run them on the
            # otherwise-idle GpSimd Q7 cores to keep ACT/DVE for psum evac
            nc.gpsimd.tensor_copy(dst, src)

        def main_units(b, segs):
            """Yield 16 closures (one per P=100 window matmul + evac, with the
            scale's store attached to its last unit)."""
            y0 = yp.tile([100, 4, 2, D], BF16, name="y0", tag="y0")
            y1 = yp.tile([100, 4, 2, D], BF16, name="y1", tag="y1")
            y2 = yp.tile([100, 2, 2, D], BF16, name="y2", tag="y2")
            y3 = yp.tile([80, 1, 2, D], BF16, name="y3", tag="y3")
            y4 = yp.tile([96, 1, 5, D], BF16, name="y4", tag="y4")
            ytiles = [y0, y1, y2, y3, y4]

            # r=1: 8 (j, c) units, stride-2 stationary -> rows 200j+2p+c
            for j in range(4):
                for c in range(2):

                    def unit(j=j, c=c):
                        ecnt[0] = 2 * j + c
                        mp = mpsp.tile([100, D], F32, name="mainps", tag="mainps")
                        for k, (d0, dw) in enumerate(DSLICES):
                            lhs = segs[k][0:dw, 200 * j : 200 * (j + 1)].rearrange(
                                "p (g c) -> p c g", c=2
                            )[:, c, :]
                            nc.tensor.matmul(
                                mp[:, :], lhs, wtiles[0][k][:, :],
                                start=(k == 0), stop=(k == 2),
                            )
                        relu_to(y0[:, j, c, :], mp[:, :])
                        if j == 3 and c == 1:
                            emit_stores(0, y0, b)

                    yield unit
            # r>=2: contiguous windows; replicas via cheap bf16 copies
            for n, c0s, p0, p1 in (
                (1, [800, 900, 1000, 1100], 0, 100),
                (2, [1200, 1300], 0, 100),
                (3, [1400], 0, 80),
                (4, [1416], 64, 96),
            ):
                for j, c0 in enumerate(c0s):

                    def unit(n=n, j=j, c0=c0, p0=p0, p1=p1, last=(c0 == c0s[-1])):
                        ecnt[0] = n + j
                        y = ytiles[n]
                        mp = mpsp.tile([100, D], F32, name="mainps", tag="mainps")
                        for k, (d0, dw) in enumerate(DSLICES):
                            nc.tensor.matmul(
                                mp[:, :],
                                segs[k][0:dw, c0 : c0 + 100],
                                wtiles[n][k][:, :],
                                start=(k == 0), stop=(k == 2),
                            )
                        relu_to(y[p0:p1, j, 0, :], mp[p0:p1, :])
                        copy_to(y[p0:p1, j, 1, :], y[p0:p1, j, 0, :])
                        if n == 4:
                            copy_to(
                                y[p0:p1, j, 2:4, :].rearrange("p c e -> p (c e)"),
                                y[p0:p1, j, 0:2, :].rearrange("p c e -> p (c e)"),
                            )
                            copy_to(y[p0:p1, j, 4, :], y[p0:p1, j, 0, :])
                        if last:
                            emit_stores(n, y, b)

                    yield unit

        def emit_stores(n, y, b):
            # alternate store issue between the SP (sync) and GpSimd DMA
            # queues; every descriptor is >=1200B of contiguous DRAM
            r = R_SCALES[n]
            dst = out_ap[n, b]
            # the gpsimd (SWDGE) ring hits periodic full-drain barriers, so
            # it carries only loads in steady state; all stores ride the sync
            # HWDGE ring except the final batch, which splits across both so
            # the tail drains twice as fast
            engs = [nc.sync, nc.gpsimd] if b >= B_LOC - 2 else [nc.sync]
            if n == 0:
                engs[b % len(engs)].dma_start(
                    out=dst[:, :].rearrange("(j p c) e -> p j (c e)", p=100, c=2),
                    in_=y[:, :, :, :].rearrange("p j c e -> p j (c e)"),
                )
                return
            rho = 5 if n == 4 else 2
            q = r // rho
            p0, p1 = {1: (0, 100), 2: (0, 100), 3: (0, 80), 4: (64, 96)}[n]
            pw = p1 - p0
            nj = y.shape[1]
            for j in range(nj):
                src = y[p0:p1, j, :, :].rearrange("p c e -> p (c e)")
                lo = (j * 100) * r
                if q > 1:
                    src = src.unsqueeze(1).to_broadcast((pw, q, rho * D))
                    dst_ = dst[lo : lo + pw * r].rearrange(
                        "(p q c) e -> p q (c e)", q=q, c=rho
                    )
                else:
                    dst_ = dst[lo : lo + pw * r].rearrange(
                        "(p c) e -> p (c e)", c=rho
                    )
                engs[(b + j + n) % len(engs)].dma_start(out=dst_, in_=src)

        # software pipeline: segsum of batch b interleaved with main of b-1;
        # FLUSH[i] main units after each of the 6 segsum chunks keep the PE
        # stream dense.  h-major chunk order lets batch 0 start on the first
        # half-load of x.  DMA rings are in-order, so loads are queued in
        # need-time order: gpsimd [spk, x0a, ...], sync [x0b, weights, ...],
        # and each x prefetch is emitted mid-batch.
        prev_units = None
        x_cur = load_x(0, split=True)
        for n in range(5):  # weights per scale, in unit order (r=1 first)
            nc.gpsimd.dma_start(out=wall[:, n, :, :], in_=wt_ap[:, n, :, :])
        for b in range(B_LOC):
            x_next = None
            segs = [
                segp.tile([101, GTOT + GPAD], BF16, name=f"seg{k}", tag=f"seg{k}")
                for k in range(3)
            ]
            for k in range(3):
                # the 4 pad cols are read (never used) by the r25 window matmul
                nc.vector.memset(segs[k][0:101, GTOT : GTOT + GPAD], 0.0)
            ci = 0
            for h in range(NH):
                for k in range(3):
                    segsum_chunk(x_cur, segs, k, h)
                    if prev_units is not None:
                        for _ in range(FLUSH[ci]):
                            u = next(prev_units, None)
                            if u is not None:
                                u()
                    ci += 1
                    if ci == 3 and b + 1 < B_LOC:
                        x_next = load_x(b + 1)
            prev_units = main_units(b, segs)
            x_cur = x_next
        for u in prev_units:
            u()


def build_module():
    nc = bacc.Bacc("TRN2", target_bir_lowering=False, debug=False)
    x = nc.dram_tensor("x", [B_LOC, LT, NT, 304], BF16, kind="ExternalInput")
    wt = nc.dram_tensor("wt", [101, 5, 3, D], BF16, kind="ExternalInput")
    spk = nc.dram_tensor("spk", [LT, NT, SCP], BF16, kind="ExternalInput")
    out = nc.dram_tensor("out", [5, B_LOC, L, D], BF16, kind="ExternalOutput")
    with tile.TileContext(nc) as tc:
        _body(tc, out.ap(), x.ap(), wt.ap(), spk.ap())
    nc.compile()
    return nc


_MODULE = None


def _get_module():
    global _MODULE
    if _MODULE is None:
        _MODULE = build_module()
    return _MODULE


def make_in_maps(inputs_c_e, W, b):
    x = np.asarray(inputs_c_e, np.float32)
    x_aug = np.concatenate(
        [x, np.ones((B, L, 1), np.float32)], axis=2
    ).astype(ml_dtypes.bfloat16)  # [B, 800, 301]
    # pack p-major: partition p holds l-rows {100t+p}, contiguous 4864B
    xpk = np.zeros((B, LT, NT, 304), ml_dtypes.bfloat16)
    xpk[:, :, :, 0:301] = x_aug.reshape(B, NT, LT, 301).transpose(0, 2, 1, 3)
    wt = build_wt_aug(W, b)
    spk = build_s_pack()
    # staging order follows dict order: constants first so the first
    # segsum chunk and batch-0 main units aren't blocked on H2D staging
    return [
        {
            "wt": wt,
            "spk": spk,
            "x": np.ascontiguousarray(xpk[c * B_LOC : (c + 1) * B_LOC]),
        }
        for c in range(NCORES)
    ]


def kernel(inputs_c_e, W, b):
    nc = _get_module()
    in_maps = make_in_maps(inputs_c_e, W, b)
    res = run_bass_kernel_spmd(nc, in_maps, core_ids=list(range(NCORES)))
    out = np.empty((5, B, L, D), np.float32)
    for c in range(NCORES):
        out[:, c * B_LOC : (c + 1) * B_LOC] = res.results[c]["out"]
    return out


# revision 25
# speedup vs baseline: 1.1174x; 1.1174x over previous
"""Trainium2 Bass kernel for nn_ContractExpand (segment_reduce, 5 scales).

out[n, b, l, e] = relu(segsum_r(x)[b, g(l), :] @ (W[n]/r).T + b[n]/r)  broadcast over groups

Strategy (data-parallel over B across 8 cores, 8 batches each):
 - host: augment x with a ones-column plus zero pad, packed p-major as
   [B_LOC,100,8,304] so every load descriptor is 4864B contiguous; fold
   bias + the 1/r scale into WT_aug[n] = [W[n].T/r ; b[n]/r^2 ; 0 ; 0],
   shipped pre-transposed as [3,101,5,300] (3000B descriptors).
 - the PE stalls ~300ns whenever consecutive matmuls change the stationary's
   32-aligned row/col-group footprint (measured), so every matmul keeps
   K in {100,101} and P=100/101 (4x4 groups).  Strided stationary reads are
   free (measured), which the r=1 path uses to pair-interleave output rows.
 - device, per batch:
     1. segsum-matmul: PE computes seg_augT[d, g] for ALL 5 scales at once
        into tile-major PSUM, two ping-pong halves (4 l-tiles each).
     2. de-scramble PSUM -> SBUF bf16 per (half, scale) on DVE/ACT/GpSimd.
     3. main matmul in P=100 g-tile windows (r=10/25 use overlapping
        windows so P stays 100); ReLU evacuates to SBUF bf16.
     4. stores: every descriptor >= 1200B of contiguous DRAM (sub-KB
        descriptors collapse SDMA throughput, measured):
          r=1  -> rows pair-interleaved into partitions via stride-2
                  stationary cols (2 consecutive rows per partition)
          r>=2 -> rho in-SBUF row replicas (rho=2, r=25: 5) written by
                  cheap bf16 copies; DMA broadcast covers only r/rho.
   Host upcasts bf16 -> f32.
"""

import numpy as np
import ml_dtypes

import concourse.bass as bass
import concourse.tile as tile
from concourse import bacc, mybir
from concourse.bass_utils import run_bass_kernel_spmd

F32 = mybir.dt.float32
BF16 = mybir.dt.bfloat16

R_SCALES = (1, 2, 4, 10, 25)
B, L, D = 64, 800, 300
NCORES = 8
B_LOC = B // NCORES          # 8 batches per core
LT = 100                     # l-tile rows; all scale group sizes align
NT = L // LT                 # 8 l-tiles
NH = 2                       # segsum psum halves (ping-pong)
TPH = NT // NH               # l-tiles per half
SCOLS = [LT // r for r in R_SCALES]                 # 100 50 25 10 4
SCOFF = np.cumsum([0] + SCOLS).tolist()             # s_pack col offsets
SC = SCOFF[-1]                                      # 189
SCP = 190                                           # padded (even) s_pack cols
G = [L // r for r in R_SCALES]                      # 800 400 200 80 32
POFF = np.cumsum([0] + G).tolist()                  # packed seg col offsets
GTOT = POFF[-1]                                     # 1512
GPAD = 4                     # seg pad cols so the r25 window stays in-tile
DSLICES = [(0, 101), (101, 101), (202, 101)]        # equal K-tiles of x_aug
PBLK = 256                                          # psum cols per l-tile block
FLUSH = [3, 3, 3, 3, 2, 2]   # units flushed after each of 6 segsum chunks


def build_s_pack():
    s = np.zeros((LT, NT, SCP), np.float32)
    for t in range(NT):
        for si, r in enumerate(R_SCALES):
            for p in range(LT):
                s[p, t, SCOFF[si] + p // r] = 1.0
    return s.astype(ml_dtypes.bfloat16)


def build_wt_aug(W, b):
    out = np.zeros((5, 303, D), np.float64)
    for n, r in enumerate(R_SCALES):
        out[n, :D, :] = np.asarray(W[n], np.float64).T / r
        out[n, D, :] = np.asarray(b[n], np.float64) / (r * r)
    # host layout matches the SBUF tile ([101, 5, 3, 300], scale-major) so
    # each per-scale load is a flat copy with 1800B-contiguous descriptors
    w3 = out.astype(ml_dtypes.bfloat16).transpose(1, 0, 2).reshape(3, 101, 5, D)
    return np.ascontiguousarray(w3.transpose(1, 2, 0, 3))


def _body(tc, out_ap, x_ap, wt_ap, spk_ap):
    nc = tc.nc
    with (
        tc.tile_pool(name="consts", bufs=1) as consts,
        tc.tile_pool(name="xp", bufs=3) as xp,
        tc.tile_pool(name="segp", bufs=2) as segp,
        tc.tile_pool(name="yp", bufs=3) as yp,
        tc.tile_pool(name="psp", bufs=2, space="PSUM") as psp,
        tc.tile_pool(name="mpsp", bufs=4, space="PSUM") as mpsp,
    ):
        spk_sb = consts.tile([LT, NT, SCP], BF16, name="spk_sb")
        nc.gpsimd.dma_start(out=spk_sb[:, :, :], in_=spk_ap[:, :, :])
        wall = consts.tile([101, 5, 3, D], BF16, name="wall")
        wtiles = [[wall[:, n, k, :] for k in range(3)] for n in range(5)]

        def load_x(b, split=False):
            # all loads ride the gpsimd (SWDGE) ring in need-order; the first
            # load is split so the h=0 segsum chunks start after half the data
            x_sb = xp.tile([LT, NT, 304], BF16, name="x_sb", tag="x")
            if split:
                nc.gpsimd.dma_start(out=x_sb[:, 0:TPH, :], in_=x_ap[b][:, 0:TPH, :])
                nc.scalar.dma_start(out=x_sb[:, TPH:NT, :], in_=x_ap[b][:, TPH:NT, :])
            else:
                nc.gpsimd.dma_start(out=x_sb[:, :, :], in_=x_ap[b][:, :, :])
            return x_sb

        def segsum_chunk(x_sb, segs, k, h):
            """Segsum matmuls for l-tiles [4h, 4h+4) of K-tile k, then the
            de-scramble copies into segs[k]."""
            d0, dw = DSLICES[k]
            ps = psp.tile([101, TPH * PBLK], F32, name="segps", tag="segps")
            for i in range(TPH):
                t = TPH * h + i
                dst = PBLK * i
                nc.tensor.matmul(
                    ps[0:dw, dst : dst + SC],
                    x_sb[:, t, d0 : d0 + dw],
                    spk_sb[:, t, 0:SC],
                    start=(i % 2 == 0),
                    stop=(i % 2 == 1),
                )
            pst = ps[0:dw, :].rearrange("p (t c) -> p t c", c=PBLK)
            for si in range(5):
                w_ = SCOLS[si]
                src = pst[:, :, SCOFF[si] : SCOFF[si] + w_]
                c0 = POFF[si] + h * TPH * w_
                dst_ = segs[k][0:dw, c0 : c0 + TPH * w_].rearrange(
                    "p (t c) -> p t c", t=TPH
                )
                # balance: big copy on DVE, the rest on ACT
                if si in (0, 4):
                    nc.vector.tensor_copy(dst_, src)
                else:
                    nc.scalar.copy(dst_, src)

        ecnt = [0]

        def eng_op(fn_dve, fn_act):
            if ecnt[0] % 2 == 0:
                fn_dve()
            else:
                fn_act()
            ecnt[0] += 1

        def relu_to(y_slice, mp):
            eng_op(
                lambda: nc.vector.tensor_scalar_max(y_slice, mp, 0.0),
                lambda: nc.scalar.activation(
                    y_slice, mp, mybir.ActivationFunctionType.Relu
                ),
            )

        def copy_to(dst, src):
            eng_op(
                lambda: nc.vector.tensor_copy(dst, src),
                lambda: nc.scalar.copy(dst, src),
            )

        def main_units(b, segs):
            """Yield 16 closures (one per P=100 window matmul + evac, with the
            scale's store attached to its last unit)."""
            y0 = yp.tile([100, 4, 2, D], BF16, name="y0", tag="y0")
            y1 = yp.tile([100, 4, 2, D], BF16, name="y1", tag="y1")
            y2 = yp.tile([100, 2, 2, D], BF16, name="y2", tag="y2")
            y3 = yp.tile([80, 1, 2, D], BF16, name="y3", tag="y3")
            y4 = yp.tile([96, 1, 5, D], BF16, name="y4", tag="y4")
            ytiles = [y0, y1, y2, y3, y4]

            # r=1: 8 (j, c) units, stride-2 stationary -> rows 200j+2p+c
            for j in range(4):
                for c in range(2):

                    def unit(j=j, c=c):
                        ecnt[0] = 2 * j + c
                        mp = mpsp.tile([100, D], F32, name="mainps", tag="mainps")
                        for k, (d0, dw) in enumerate(DSLICES):
                            lhs = segs[k][0:dw, 200 * j : 200 * (j + 1)].rearrange(
                                "p (g c) -> p c g", c=2
                            )[:, c, :]
                            nc.tensor.matmul(
                                mp[:, :], lhs, wtiles[0][k][:, :],
                                start=(k == 0), stop=(k == 2),
                            )
                        relu_to(y0[:, j, c, :], mp[:, :])
                        if j == 3 and c == 1:
                            emit_stores(0, y0, b)

                    yield unit
            # r>=2: contiguous windows; replicas via cheap bf16 copies
            for n, c0s, p0, p1 in (
                (1, [800, 900, 1000, 1100], 0, 100),
                (2, [1200, 1300], 0, 100),
                (3, [1400], 0, 80),
                (4, [1416], 64, 96),
            ):
                for j, c0 in enumerate(c0s):

                    def unit(n=n, j=j, c0=c0, p0=p0, p1=p1, last=(c0 == c0s[-1])):
                        ecnt[0] = n + j
                        y = ytiles[n]
                        mp = mpsp.tile([100, D], F32, name="mainps", tag="mainps")
                        for k, (d0, dw) in enumerate(DSLICES):
                            nc.tensor.matmul(
                                mp[:, :],
                                segs[k][0:dw, c0 : c0 + 100],
                                wtiles[n][k][:, :],
                                start=(k == 0), stop=(k == 2),
                            )
                        relu_to(y[p0:p1, j, 0, :], mp[p0:p1, :])
                        copy_to(y[p0:p1, j, 1, :], y[p0:p1, j, 0, :])
                        if n == 4:
                            copy_to(
                                y[p0:p1, j, 2:4, :].rearrange("p c e -> p (c e)"),
                                y[p0:p1, j, 0:2, :].rearrange("p c e -> p (c e)"),
                            )
                            copy_to(y[p0:p1, j, 4, :], y[p0:p1, j, 0, :])
                        if last:
                            emit_stores(n, y, b)

                    yield unit

        def emit_stores(n, y, b):
            # alternate store issue between the SP (sync) and GpSimd DMA
            # queues; every descriptor is >=1200B of contiguous DRAM
            r = R_SCALES[n]
            dst = out_ap[n, b]
            # the gpsimd (SWDGE) ring hits periodic full-drain barriers, so
            # it carries only loads in steady state; all stores ride the sync
            # HWDGE ring except the final batch, which splits across both so
            # the tail drains twice as fast
            engs = [nc.sync, nc.gpsimd] if b >= B_LOC - 2 else [nc.sync]
            if n == 0:
                engs[b % len(engs)].dma_start(
                    out=dst[:, :].rearrange("(j p c) e -> p j (c e)", p=100, c=2),
                    in_=y[:, :, :, :].rearrange("p j c e -> p j (c e)"),
                )
                return
            rho = 5 if n == 4 else 2
            q = r // rho
            p0, p1 = {1: (0, 100), 2: (0, 100), 3: (0, 80), 4: (64, 96)}[n]
            pw = p1 - p0
            nj = y.shape[1]
            for j in range(nj):
                src = y[p0:p1, j, :, :].rearrange("p c e -> p (c e)")
                lo = (j * 100) * r
                if q > 1:
                    src = src.unsqueeze(1).to_broadcast((pw, q, rho * D))
                    dst_ = dst[lo : lo + pw * r].rearrange(
                        "(p q c) e -> p q (c e)", q=q, c=rho
                    )
                else:
                    dst_ = dst[lo : lo + pw * r].rearrange(
                        "(p c) e -> p (c e)", c=rho
                    )
                engs[(b + j + n) % len(engs)].dma_start(out=dst_, in_=src)

        # software pipeline: segsum of batch b interleaved with main of b-1;
        # FLUSH[i] main units after each of the 6 segsum chunks keep the PE
        # stream dense.  h-major chunk order lets batch 0 start on the first
        # half-load of x.  DMA rings are in-order, so loads are queued in
        # need-time order: gpsimd [spk, x0a, ...], sync [x0b, weights, ...],
        # and each x prefetch is emitted mid-batch.
        prev_units = None
        x_cur = load_x(0, split=True)
        for n in range(5):  # weights per scale, in unit order (r=1 first)
            nc.gpsimd.dma_start(out=wall[:, n, :, :], in_=wt_ap[:, n, :, :])
        for b in range(B_LOC):
            x_next = None
            segs = [
                segp.tile([101, GTOT + GPAD], BF16, name=f"seg{k}", tag=f"seg{k}")
                for k in range(3)
            ]
            for k in range(3):
                # the 4 pad cols are read (never used) by the r25 window matmul
                nc.vector.memset(segs[k][0:101, GTOT : GTOT + GPAD], 0.0)
            ci = 0
            for h in range(NH):
                for k in range(3):
                    segsum_chunk(x_cur, segs, k, h)
                    if prev_units is not None:
                        for _ in range(FLUSH[ci]):
                            u = next(prev_units, None)
                            if u is not None:
                                u()
                    ci += 1
                    if ci == 3 and b + 1 < B_LOC:
                        x_next = load_x(b + 1)
            prev_units = main_units(b, segs)
            x_cur = x_next
        for u in prev_units:
            u()


def build_module():
    nc = bacc.Bacc("TRN2", target_bir_lowering=False, debug=False)
    x = nc.dram_tensor("x", [B_LOC, LT, NT, 304], BF16, kind="ExternalInput")
    wt = nc.dram_tensor("wt", [101, 5, 3, D], BF16, kind="ExternalInput")
    spk = nc.dram_tensor("spk", [LT, NT, SCP], BF16, kind="ExternalInput")
    out = nc.dram_tensor("out", [5, B_LOC, L, D], BF16, kind="ExternalOutput")
    with tile.TileContext(nc) as tc:
        _body(tc, out.ap(), x.ap(), wt.ap(), spk.ap())
    nc.compile()
    return nc


_MODULE = None


def _get_module():
    global _MODULE
    if _MODULE is None:
        _MODULE = build_module()
    return _MODULE


def make_in_maps(inputs_c_e, W, b):
    x = np.asarray(inputs_c_e, np.float32)
    x_aug = np.concatenate(
        [x, np.ones((B, L, 1), np.float32)], axis=2
    ).astype(ml_dtypes.bfloat16)  # [B, 800, 301]
    # pack p-major: partition p holds l-rows {100t+p}, contiguous 4864B
    xpk = np.zeros((B, LT, NT, 304), ml_dtypes.bfloat16)
    xpk[:, :, :, 0:301] = x_aug.reshape(B, NT, LT, 301).transpose(0, 2, 1, 3)
    wt = build_wt_aug(W, b)
    spk = build_s_pack()
    # staging order follows dict order: constants first so the first
    # segsum chunk and batch-0 main units aren't blocked on H2D staging
    return [
        {
            "wt": wt,
            "spk": spk,
            "x": np.ascontiguousarray(xpk[c * B_LOC : (c + 1) * B_LOC]),
        }
        for c in range(NCORES)
    ]


def kernel(inputs_c_e, W, b):
    nc = _get_module()
    in_maps = make_in_maps(inputs_c_e, W, b)
    res = run_bass_kernel_spmd(nc, in_maps, core_ids=list(range(NCORES)))
    out = np.empty((5, B, L, D), np.float32)
    for c in range(NCORES):
        out[:, c * B_LOC : (c + 1) * B_LOC] = res.results[c]["out"]
    return out
